# revision 19
# baseline (speedup 1.0000x reference)
"""Trainium2 Bass kernel for nn_ChoquetIntegralConstrained.

Computes: sigmoid((x @ w_eff) / weight_sum - thr) where w_eff is built from
(wc, wint) via the constraint transform, x is [16384, 8256] f32.

Strategy: pure data parallel over batch across 8 NeuronCores. Each core gets
2048 rows, processed as 16 tiles of [128 rows, 8256], each arriving as four
quarter-chunk DMAs spread over both HWDGE rings. The fused multiply+row-reduce
is one fused DVE scalar_tensor_tensor per quarter (out = x*w via a stride-0
dummy, accum_out = row-sum). Offloading quarters to Pool/ACT was tried and
reverted: concurrent Pool reads of the x stream slow the DVE STTs ~2.8x via
SBUF contention, a net loss. DVE consumes slightly faster than the two DMA
rings deliver, so the kernel rides the HBM roofline. The tiny constraint transform on the 8256 weights is done on the
host in fp32 (identical elementwise semantics to the reference).
"""

import sys

import numpy as np

sys.path.insert(0, "/opt/trn_rl_repo")

N_CRIT = 128
N_PAIRS = N_CRIT * (N_CRIT - 1) // 2  # 8128
D = N_CRIT + N_PAIRS  # 8256
BATCH = 16384
N_CORES = 8
ROWS_PER_CORE = BATCH // N_CORES  # 2048
P = 128  # SBUF partitions
TILES_PER_CORE = ROWS_PER_CORE // P  # 16
MIN_W = np.float32(1e-07)

_CACHE = {}


def _build_program():
    import concourse.tile as tile
    from concourse import bacc, mybir

    nc = bacc.Bacc(
        "TRN2",
        debug=False,
        target_bir_lowering=False,
        num_devices=N_CORES,
    )
    f32 = mybir.dt.float32
    x_d = nc.dram_tensor("x", [ROWS_PER_CORE, D], f32, kind="ExternalInput").ap()
    w_d = nc.dram_tensor("w128", [P, D], f32, kind="ExternalInput").ap()
    c_d = nc.dram_tensor("consts", [P, 2], f32, kind="ExternalInput").ap()
    y_d = nc.dram_tensor("y", [P, TILES_PER_CORE], f32, kind="ExternalOutput").ap()

    CH = D // 4  # 2064

    with tile.TileContext(nc) as tc:
        with (
            tc.tile_pool(name="xp", bufs=4) as xp,
            tc.tile_pool(name="xcp", bufs=4) as xcp,
            tc.tile_pool(name="wp", bufs=1) as wp,
        ):
            c_t = wp.tile([P, 2], f32)
            nc.gpsimd.dma_start(out=c_t[:], in_=c_d[:])
            # w arrives pre-replicated from the host as a [128, D] input and
            # is DMA'd in 4 contiguous quarters over the Pool SWDGE queue
            # (+4MB on the 68MB DMA budget). This keeps PE/ACT off the
            # critical path entirely (a PE fp32 matmul broadcast costs ~55us
            # of LOW/HIGH passes and serializes the scalar ring's x
            # doorbells behind its ACT copies; a stride-0-source SWDGE
            # broadcast spends ~47us in Q7 descriptor generation) and the
            # HWDGE rings carry nothing but x. w stays in 4 quarter tiles so
            # a quarter-q STT only waits for its own quarter's DMA.
            w_quarters = [
                wp.tile([P, CH], f32, name=f"w_q{q}") for q in range(4)
            ]
            for q in range(4):
                nc.gpsimd.dma_start(
                    out=w_quarters[q][:],
                    in_=w_d[:, q * CH : (q + 1) * CH],
                )

            acc_t = wp.tile([P, TILES_PER_CORE], f32)
            accq_v = wp.tile([P, TILES_PER_CORE * 4], f32)
            # STT must write a full-size out; a stride-0 broadcast AP over a
            # [P, 1] dummy absorbs it without SBUF cost.
            dummy_v = wp.tile([P, 1], f32)
            y_t = wp.tile([P, TILES_PER_CORE], f32)

            # x DMAs alternate between the two HWDGE rings (ACT and SP).
            # Middle tiles move as single full-tile DMAs - 33KB-per-
            # partition descriptor lines sustain ~420GB/s where 8KB quarter
            # chunks cap out ~15-20% lower on per-descriptor overhead. Only
            # the edge tiles are quartered: the first two so the DVE starts
            # ~8us in, the last two so the end-of-kernel tail is one
            # quarter-STT instead of a full-tile backlog.
            dma_engines = (nc.scalar, nc.sync)
            n_dma = 0
            EDGE = {0, 1, TILES_PER_CORE - 2, TILES_PER_CORE - 1}

            # Preload the ACT sigmoid table early so the final activation
            # doesn't eat a ~1.3us table load on the critical tail.
            nc.scalar.activation(
                out=y_t[:, 0:1],
                in_=c_t[:, 0:1],
                func=mybir.ActivationFunctionType.Sigmoid,
                bias=c_t[:, 1:2],
                scale=c_t[:, 0:1],
            )

            def quarter_stt(src_ap, t, q):
                col = 4 * t + q
                nc.vector.scalar_tensor_tensor(
                    out=dummy_v.broadcast_to((P, CH)),
                    in0=src_ap,
                    scalar=1.0,
                    in1=w_quarters[q][:],
                    op0=mybir.AluOpType.mult,
                    op1=mybir.AluOpType.mult,
                    accum_out=accq_v[:, col : col + 1],
                )

            for t in range(TILES_PER_CORE):
                rows = slice(t * P, (t + 1) * P)
                if t in EDGE:
                    for q in range(4):
                        x_c = xcp.tile([P, CH], f32, tag="x_c")
                        dma_engines[n_dma % 2].dma_start(
                            out=x_c[:], in_=x_d[rows, q * CH : (q + 1) * CH]
                        )
                        n_dma += 1
                        quarter_stt(x_c[:], t, q)
                else:
                    x_t = xp.tile([P, D], f32, tag="x_t")
                    dma_engines[n_dma % 2].dma_start(out=x_t[:], in_=x_d[rows, :])
                    n_dma += 1
                    for q in range(4):
                        quarter_stt(x_t[:, q * CH : (q + 1) * CH], t, q)

            # Combine the per-quarter partial sums of every tile.
            nc.vector.tensor_reduce(
                out=acc_t[:],
                in_=accq_v[:].rearrange("p (t q) -> p t q", q=4),
                axis=mybir.AxisListType.X,
                op=mybir.AluOpType.add,
            )

            nc.scalar.activation(
                out=y_t[:],
                in_=acc_t[:],
                func=mybir.ActivationFunctionType.Sigmoid,
                bias=c_t[:, 1:2],
                scale=c_t[:, 0:1],
            )
            nc.sync.dma_start(out=y_d[:], in_=y_t[:])

    nc.compile()
    return nc


def _get_program():
    if "nc" not in _CACHE:
        _CACHE["nc"] = _build_program()
    return _CACHE["nc"]


def _host_weight_prep(wc, wint, thr):
    """Mirror reference._constrained_weights + weight_sum in fp32 numpy."""
    wc = np.asarray(wc, dtype=np.float32)
    wint = np.asarray(wint, dtype=np.float32)
    wc_eff = np.where(wc < 0, MIN_W, wc)
    ii, jj = np.triu_indices(N_CRIT, k=1)
    lower = np.maximum(-wc_eff[:, ii], -wc_eff[:, jj])
    wint_eff = np.maximum(wint, lower)
    w_eff = np.concatenate([wc_eff, wint_eff], axis=1)  # [1, D]
    wsum = np.float32(wc_eff.sum(dtype=np.float32)) + np.float32(
        wint_eff.sum(dtype=np.float32)
    )
    inv_wsum = np.float32(1.0) / wsum
    neg_thr = -np.float32(np.asarray(thr).reshape(-1)[0])
    return w_eff, inv_wsum, neg_thr


def _make_in_maps(x, wc, wint, thr):
    x = np.ascontiguousarray(np.asarray(x, dtype=np.float32))
    w_eff, inv_wsum, neg_thr = _host_weight_prep(wc, wint, thr)
    w128 = np.ascontiguousarray(np.broadcast_to(w_eff, (P, D)))
    consts = np.empty((P, 2), dtype=np.float32)
    consts[:, 0] = inv_wsum
    consts[:, 1] = neg_thr
    return [
        {
            "x": np.ascontiguousarray(x[c * ROWS_PER_CORE : (c + 1) * ROWS_PER_CORE]),
            "w128": w128,
            "consts": consts,
        }
        for c in range(N_CORES)
    ]


def _gather(results):
    # y core tile is [P, TILES]: y[p, t] = batch row t*128 + p within the shard
    parts = [
        np.asarray(results[c]["y"]).T.reshape(ROWS_PER_CORE) for c in range(N_CORES)
    ]
    return np.concatenate(parts).reshape(BATCH, 1).astype(np.float32)


def _run(x, wc, wint, thr, trace=False):
    from concourse import bass_utils

    nc = _get_program()
    in_maps = _make_in_maps(x, wc, wint, thr)
    res = bass_utils.run_bass_kernel_spmd(
        nc, in_maps, core_ids=list(range(N_CORES)), trace=trace
    )
    return _gather(res.results), res


def kernel(x, wc, wint, thr):
    out, _ = _run(x, wc, wint, thr, trace=False)
    return out



# revision 23
# speedup vs baseline: 1.0208x; 1.0208x over previous
"""Trainium2 Bass kernel for nn_ChoquetIntegralConstrained.

Computes: sigmoid((x @ w_eff) / weight_sum - thr) where w_eff is built from
(wc, wint) via the constraint transform, x is [16384, 8256] f32.

Strategy: pure data parallel over batch across 8 NeuronCores. Each core gets
2048 rows, processed as 16 tiles of [128 rows, 8256], each arriving as four
quarter-chunk DMAs spread over both HWDGE rings. The fused multiply+row-reduce
is one fused DVE scalar_tensor_tensor per quarter (out = x*w via a stride-0
dummy, accum_out = row-sum). Offloading quarters to Pool/ACT was tried and
reverted: concurrent Pool reads of the x stream slow the DVE STTs ~2.8x via
SBUF contention, a net loss. DVE consumes slightly faster than the two DMA
rings deliver, so the kernel rides the HBM roofline. The tiny constraint transform on the 8256 weights is done on the
host in fp32 (identical elementwise semantics to the reference).
"""

import sys

import numpy as np

sys.path.insert(0, "/opt/trn_rl_repo")

N_CRIT = 128
N_PAIRS = N_CRIT * (N_CRIT - 1) // 2  # 8128
D = N_CRIT + N_PAIRS  # 8256
BATCH = 16384
N_CORES = 8
ROWS_PER_CORE = BATCH // N_CORES  # 2048
P = 128  # SBUF partitions
TILES_PER_CORE = ROWS_PER_CORE // P  # 16
MIN_W = np.float32(1e-07)

_CACHE = {}


def _build_program():
    import concourse.tile as tile
    from concourse import bacc, mybir

    nc = bacc.Bacc(
        "TRN2",
        debug=False,
        target_bir_lowering=False,
        num_devices=N_CORES,
    )
    f32 = mybir.dt.float32
    x_d = nc.dram_tensor("x", [ROWS_PER_CORE, D], f32, kind="ExternalInput").ap()
    w_d = nc.dram_tensor("w128", [P, D], f32, kind="ExternalInput").ap()
    c_d = nc.dram_tensor("consts", [P, 2], f32, kind="ExternalInput").ap()
    y_d = nc.dram_tensor("y", [P, TILES_PER_CORE], f32, kind="ExternalOutput").ap()

    CH = D // 4  # 2064

    with tile.TileContext(nc) as tc:
        with (
            tc.tile_pool(name="xp", bufs=4) as xp,
            tc.tile_pool(name="xcp", bufs=4) as xcp,
            tc.tile_pool(name="wp", bufs=1) as wp,
        ):
            c_t = wp.tile([P, 2], f32)
            nc.gpsimd.dma_start(out=c_t[:], in_=c_d[:])
            # w arrives pre-replicated from the host as a [128, D] input,
            # DMA'd in 4 contiguous quarters at the FRONT of the two HWDGE
            # rings (+4MB on the 68MB DMA budget). Alternatives measured
            # worse: a PE fp32 matmul broadcast costs ~55us of LOW/HIGH
            # passes and serializes the scalar ring's x doorbells behind its
            # ACT copies; any [128, CH] transfer on the Pool SWDGE queue
            # spends ~12-15us in Q7 descriptor generation and drags the
            # whole DMA subsystem to ~330GB/s while it runs. w_q0/w_q1 go
            # first so the DVE can start at ~7us; w_q2/w_q3 are interleaved
            # after tile 0's chunks, just ahead of their first use.
            w_quarters = [
                wp.tile([P, CH], f32, name=f"w_q{q}") for q in range(4)
            ]
            nc.scalar.dma_start(out=w_quarters[0][:], in_=w_d[:, 0:CH])
            nc.sync.dma_start(out=w_quarters[1][:], in_=w_d[:, CH : 2 * CH])

            acc_t = wp.tile([P, TILES_PER_CORE], f32)
            accq_v = wp.tile([P, TILES_PER_CORE * 4], f32)
            # STT must write a full-size out; a stride-0 broadcast AP over a
            # [P, 1] dummy absorbs it without SBUF cost.
            dummy_v = wp.tile([P, 1], f32)
            y_t = wp.tile([P, TILES_PER_CORE], f32)

            # x DMAs alternate between the two HWDGE rings (ACT and SP).
            # Middle tiles move as single full-tile DMAs - 33KB-per-
            # partition descriptor lines sustain ~430GB/s where 8KB quarter
            # chunks cap out ~15-20% lower on per-descriptor overhead. The
            # edge tiles are quartered and striped across both rings: tiles
            # 0-3 so arrivals track DVE demand from ~7us (full tiles would
            # land in 19.6us ring-pair beats the DVE - which has only ~6us
            # of slack vs the DMA - can never recover from), tiles 14-15 so
            # the end-of-kernel tail is one quarter-STT, not a full tile.
            dma_engines = (nc.scalar, nc.sync)
            n_dma = 0
            EDGE = {0, 1, 2, 3, TILES_PER_CORE - 2, TILES_PER_CORE - 1}

            # Preload the ACT sigmoid table early so the final activation
            # doesn't eat a ~1.3us table load on the critical tail.
            nc.scalar.activation(
                out=y_t[:, 0:1],
                in_=c_t[:, 0:1],
                func=mybir.ActivationFunctionType.Sigmoid,
                bias=c_t[:, 1:2],
                scale=c_t[:, 0:1],
            )

            def quarter_stt(src_ap, t, q):
                col = 4 * t + q
                nc.vector.scalar_tensor_tensor(
                    out=dummy_v.broadcast_to((P, CH)),
                    in0=src_ap,
                    scalar=1.0,
                    in1=w_quarters[q][:],
                    op0=mybir.AluOpType.mult,
                    op1=mybir.AluOpType.mult,
                    accum_out=accq_v[:, col : col + 1],
                )

            for t in range(TILES_PER_CORE):
                rows = slice(t * P, (t + 1) * P)
                if t in EDGE:
                    chunks = []
                    for q in range(4):
                        x_c = xcp.tile([P, CH], f32, tag="x_c")
                        dma_engines[n_dma % 2].dma_start(
                            out=x_c[:], in_=x_d[rows, q * CH : (q + 1) * CH]
                        )
                        n_dma += 1
                        chunks.append(x_c)
                    if t == 0:
                        # Issued after tile 0's chunk doorbells (ring
                        # position) but before any STT that reads them
                        # (trace order - an STT traced before the DMA write
                        # would read garbage and turn the DMA into a WAR).
                        nc.scalar.dma_start(
                            out=w_quarters[2][:], in_=w_d[:, 2 * CH : 3 * CH]
                        )
                        nc.sync.dma_start(
                            out=w_quarters[3][:], in_=w_d[:, 3 * CH : 4 * CH]
                        )
                    for q in range(4):
                        quarter_stt(chunks[q][:], t, q)
                else:
                    x_t = xp.tile([P, D], f32, tag="x_t")
                    dma_engines[n_dma % 2].dma_start(out=x_t[:], in_=x_d[rows, :])
                    n_dma += 1
                    for q in range(4):
                        quarter_stt(x_t[:, q * CH : (q + 1) * CH], t, q)

            # Combine the per-quarter partial sums of every tile.
            nc.vector.tensor_reduce(
                out=acc_t[:],
                in_=accq_v[:].rearrange("p (t q) -> p t q", q=4),
                axis=mybir.AxisListType.X,
                op=mybir.AluOpType.add,
            )

            nc.scalar.activation(
                out=y_t[:],
                in_=acc_t[:],
                func=mybir.ActivationFunctionType.Sigmoid,
                bias=c_t[:, 1:2],
                scale=c_t[:, 0:1],
            )
            nc.sync.dma_start(out=y_d[:], in_=y_t[:])

    nc.compile()
    return nc


def _get_program():
    if "nc" not in _CACHE:
        _CACHE["nc"] = _build_program()
    return _CACHE["nc"]


def _host_weight_prep(wc, wint, thr):
    """Mirror reference._constrained_weights + weight_sum in fp32 numpy."""
    wc = np.asarray(wc, dtype=np.float32)
    wint = np.asarray(wint, dtype=np.float32)
    wc_eff = np.where(wc < 0, MIN_W, wc)
    ii, jj = np.triu_indices(N_CRIT, k=1)
    lower = np.maximum(-wc_eff[:, ii], -wc_eff[:, jj])
    wint_eff = np.maximum(wint, lower)
    w_eff = np.concatenate([wc_eff, wint_eff], axis=1)  # [1, D]
    wsum = np.float32(wc_eff.sum(dtype=np.float32)) + np.float32(
        wint_eff.sum(dtype=np.float32)
    )
    inv_wsum = np.float32(1.0) / wsum
    neg_thr = -np.float32(np.asarray(thr).reshape(-1)[0])
    return w_eff, inv_wsum, neg_thr


def _make_in_maps(x, wc, wint, thr):
    x = np.ascontiguousarray(np.asarray(x, dtype=np.float32))
    w_eff, inv_wsum, neg_thr = _host_weight_prep(wc, wint, thr)
    w128 = np.ascontiguousarray(np.broadcast_to(w_eff, (P, D)))
    consts = np.empty((P, 2), dtype=np.float32)
    consts[:, 0] = inv_wsum
    consts[:, 1] = neg_thr
    return [
        {
            "x": np.ascontiguousarray(x[c * ROWS_PER_CORE : (c + 1) * ROWS_PER_CORE]),
            "w128": w128,
            "consts": consts,
        }
        for c in range(N_CORES)
    ]


def _gather(results):
    # y core tile is [P, TILES]: y[p, t] = batch row t*128 + p within the shard
    parts = [
        np.asarray(results[c]["y"]).T.reshape(ROWS_PER_CORE) for c in range(N_CORES)
    ]
    return np.concatenate(parts).reshape(BATCH, 1).astype(np.float32)


def _run(x, wc, wint, thr, trace=False):
    from concourse import bass_utils

    nc = _get_program()
    in_maps = _make_in_maps(x, wc, wint, thr)
    res = bass_utils.run_bass_kernel_spmd(
        nc, in_maps, core_ids=list(range(N_CORES)), trace=trace
    )
    return _gather(res.results), res


def kernel(x, wc, wint, thr):
    out, _ = _run(x, wc, wint, thr, trace=False)
    return out

